# revision 12
# baseline (speedup 1.0000x reference)
"""TGCN (dense-graph GRU) Trainium2 kernel, 8-core SPMD, no collectives.

Math (per reference):
  xh_t = relu(x_t @ fc_w + fc_b)                    [N, H]
  S_t  = adj @ xh_t                                 (assoc: adj@(xh@W) = (adj@xh)@W)
  z_t  = sigmoid(S_t @ Mz + h @ Lz_bot + bz)        Mz = Wz @ Lz_top (host-folded)
  r_t  = sigmoid(S_t @ Mr + h @ Lr_bot + br)
  ht_t = tanh   (S_t @ Mh + (h*r) @ Lh_bot + bh)
  h    = z*h + (1-z)*ht = h + (1-z)*(ht - h)

Sharding: row-partition adj across 8 cores (512 nodes each). The GRU cell is
row-local, so each core runs the whole time loop on its shard independently.
x is replicated (each core redundantly computes xh for all nodes).

v2 vs v1:
  - xh and S matmuls run in fp8e4 with MatmulPerfMode.DoubleRow (2 k-tiles
    per instruction, 0.5 cyc/row): S pair matmul 16x256cyc vs 32x512cyc.
    adj is scaled by N (entries would be subnormal in e4m3 otherwise) and
    fc_w by 16; both scales are folded out of the gate weights Mz/Mr/Mh.
  - Gate pre-activations use PSUM accumulation (M.T@S_t then L.T@h into the
    same bank) instead of concat tiles: no per-step concat copies, h lives
    in a plain ping-pong tile pair.
  - z and r sigmoids fused into ONE [128, NS] ACT op; the z-half weights are
    negated host-side so the op directly yields z' = 1-z alongside r.
    Combine: h_new = h + z'*(ht - h)  (3 DVE ops + 1 DVE for r*h).
  - GRU is node-chunked (C chunks) so the sequential per-step chain
    pipelines across chunks (wavefront) instead of serializing.

Dtypes: xh/S matmuls fp8e4 (DoubleRow), gate matmuls bf16, h state bf16,
PSUM f32 everywhere.
"""

import os
import sys

sys.path.insert(0, "/opt/trn_rl_repo")

import numpy as np
import ml_dtypes

T, N, F_IN, H1, F_OUT = 48, 4096, 64, 64, 64
NCORES = 8
NS = N // NCORES          # nodes per core = 512
PAIRS = T // 2            # 24
KT = N // 128             # 32 k-tiles for the adj matmul (DR: 16 instrs)
ADJ_SCALE = float(N)      # adj entries ~1/N are subnormal in fp8e4
FCW_SCALE = 16.0          # fc_w entries ~0.05 land near fp8e4 subnormals
CHUNKS = 2                # GRU node-chunking (wavefront over the chain)
CNS = NS // CHUNKS

_cache = {}


def _build():
    import concourse.bass as bass
    import concourse.mybir as mybir
    import concourse.tile as tile
    from concourse import bacc

    f32 = mybir.dt.float32
    bf16 = mybir.dt.bfloat16
    fp8 = mybir.dt.float8e4
    AF = mybir.ActivationFunctionType
    DR = mybir.MatmulPerfMode.DoubleRow
    ALU = mybir.AluOpType

    nc = bacc.Bacc(
        "TRN2",
        target_bir_lowering=False,
        debug=False,
        enable_asserts=False,
        num_devices=NCORES,
    )

    # DRAM parameters (per-core shapes)
    adjT_d = nc.dram_tensor("adjT", [128, KT, NS], fp8, kind="ExternalInput").ap()
    # x: [pair, feat%32, feat//32, step, node] (fp8, DoubleRow-packed K)
    xT_d = nc.dram_tensor("xT", [PAIRS, 32, 2, 2, N], fp8, kind="ExternalInput").ap()
    fcw_d = nc.dram_tensor("fcw", [32, 2, H1], fp8, kind="ExternalInput").ap()
    mzr_d = nc.dram_tensor("mzr", [H1, 128], bf16, kind="ExternalInput").ap()
    lzr_d = nc.dram_tensor("lzr", [F_OUT, 128], bf16, kind="ExternalInput").ap()
    mh_d = nc.dram_tensor("mh", [H1, F_OUT], bf16, kind="ExternalInput").ap()
    lh_d = nc.dram_tensor("lh", [F_OUT, F_OUT], bf16, kind="ExternalInput").ap()
    bzr_d = nc.dram_tensor("bzr", [128, 1], f32, kind="ExternalInput").ap()
    bh_d = nc.dram_tensor("bh", [F_OUT, 1], f32, kind="ExternalInput").ap()
    out_d = nc.dram_tensor("out", [F_OUT, NS], f32, kind="ExternalOutput").ap()

    with tile.TileContext(nc) as tc:
        with (
            tc.tile_pool(name="const", bufs=1) as constp,
            tc.tile_pool(name="state", bufs=1) as statep,
            tc.tile_pool(name="xt", bufs=2) as xtp,
            tc.tile_pool(name="xh", bufs=2) as xhp,
            tc.tile_pool(name="ssb", bufs=2) as ssbp,
            tc.tile_pool(name="gw", bufs=3) as gwp,
            tc.tile_pool(name="psx", bufs=2, space="PSUM") as psxp,
            tc.tile_pool(name="pss", bufs=2, space="PSUM") as pssp,
            tc.tile_pool(name="pszr", bufs=2, space="PSUM") as pszrp,
            tc.tile_pool(name="psh", bufs=2, space="PSUM") as pshp,
        ):
            # ---- constants ----
            fcw_sb = constp.tile([32, 2, H1], fp8)
            nc.sync.dma_start(out=fcw_sb[:], in_=fcw_d[:])
            adjT_sb = constp.tile([128, KT, NS], fp8)
            for q, eng in enumerate((nc.sync, nc.gpsimd, nc.gpsimd, nc.sync)):
                eng.dma_start(
                    out=adjT_sb[:, q * 8 : (q + 1) * 8, :],
                    in_=adjT_d[:, q * 8 : (q + 1) * 8, :],
                )
            mzr_sb = constp.tile([H1, 128], bf16)
            lzr_sb = constp.tile([F_OUT, 128], bf16)
            mh_sb = constp.tile([H1, F_OUT], bf16)
            lh_sb = constp.tile([F_OUT, F_OUT], bf16)
            bzr_sb = constp.tile([128, 1], f32)
            bh_sb = constp.tile([F_OUT, 1], f32)
            for dst, src in (
                (mzr_sb, mzr_d), (lzr_sb, lzr_d), (mh_sb, mh_d), (lh_sb, lh_d),
                (bzr_sb, bzr_d), (bh_sb, bh_d),
            ):
                nc.gpsimd.dma_start(out=dst[:], in_=src[:])

            # ---- state ----
            # h ping-pong: H[t % 2] holds h_{t-1} (bf16, [feat, node])
            H = [
                statep.tile([F_OUT, NS], bf16, tag=f"H{i}", name=f"H{i}")
                for i in range(2)
            ]
            nc.vector.memset(H[0][:], 0.0)

            S_tiles = [None, None]  # S_sb tile per step parity of current pair

            def emit_xh_groups(xt, xh, groups):
                # xh-pair matmuls (fp8 DoubleRow over the 2 feat-halves):
                # out[128 nodes, 64] += sum_i xt[:, i, s, blk].T @ fcw[:, i, :]
                for g in groups:
                    ps = psxp.tile([128, 512], mybir.dt.float32)
                    for j in range(4):
                        k = 4 * g + j
                        for s in (0, 1):
                            nc.tensor.matmul(
                                ps[:, j * 128 + s * 64 : j * 128 + (s + 1) * 64],
                                lhsT=xt[:, :, s, k * 128 : (k + 1) * 128],
                                rhs=fcw_sb[:],
                                start=True, stop=True,
                                perf_mode=DR,
                            )
                    # relu + f32->fp8 cast, PSUM -> SBUF. Only ACT/DVE can
                    # read PSUM (gpsimd cannot).
                    dst = xh[:, 4 * g : 4 * (g + 1), :].rearrange("p a b -> p (a b)")
                    if g % 2 == 0:
                        nc.scalar.activation(dst, ps[:], AF.Relu)
                    else:
                        nc.vector.tensor_scalar(dst, ps[:], 0.0, None, ALU.max)

            def emit_gru_front(step, c):
                # pre_zr = [-Mz|Mr].T @ S + [-Lz|Lr].T @ h  (PSUM accum),
                # sigma -> [z' | r],  rh = r * h
                s = step % 2
                sl = slice(c * CNS, (c + 1) * CNS)
                h = H[step % 2]
                ps_zr = pszrp.tile([128, CNS], mybir.dt.float32, tag=f"ps_zr{c}",
                                   bufs=1)
                nc.tensor.matmul(ps_zr[:], lhsT=mzr_sb[:], rhs=S_tiles[s][:, s, sl],
                                 start=True, stop=False)
                nc.tensor.matmul(ps_zr[:], lhsT=lzr_sb[:], rhs=h[:, sl],
                                 start=False, stop=True)
                # ZR rows 0-63: r, rows 64-127: z' = 1-z (z-weights negated)
                ZR = gwp.tile([128, CNS], bf16, tag=f"ZR{c}")
                nc.scalar.activation(ZR[:], ps_zr[:], AF.Sigmoid, bias=bzr_sb[:])
                RH = gwp.tile([F_OUT, CNS], bf16, tag=f"RH{c}")
                nc.vector.tensor_mul(RH[:], ZR[0:64, :], h[:, sl])
                return ZR, RH

            def emit_gru_back(step, c, ZR, RH):
                # pre_h = Mh.T @ S + Lh.T @ rh; ht = tanh; h += z'*(ht - h)
                s = step % 2
                sl = slice(c * CNS, (c + 1) * CNS)
                h = H[step % 2]
                hn = H[(step + 1) % 2]
                ps_h = pshp.tile([F_OUT, CNS], mybir.dt.float32, tag=f"ps_h{c}",
                                 bufs=1)
                nc.tensor.matmul(ps_h[:], lhsT=mh_sb[:], rhs=S_tiles[s][:, s, sl],
                                 start=True, stop=False)
                nc.tensor.matmul(ps_h[:], lhsT=lh_sb[:], rhs=RH[:],
                                 start=False, stop=True)
                HT = gwp.tile([F_OUT, CNS], bf16, tag=f"HT{c}")
                nc.scalar.activation(HT[:], ps_h[:], AF.Tanh, bias=bh_sb[:])
                # TensorTensor inputs must share a start partition; outputs
                # are free. D lands at base 64 to pair with z' (rows 64-127),
                # P lands back at base 0 to pair with h.
                D = gwp.tile([128, CNS], bf16, tag=f"D{c}")
                nc.vector.tensor_tensor(D[64:128, :], HT[:], h[:, sl],
                                        ALU.subtract)
                P = gwp.tile([F_OUT, CNS], bf16, tag=f"P{c}")
                nc.vector.tensor_mul(P[:], ZR[64:128, :], D[64:128, :])
                nc.vector.tensor_add(hn[:, sl], h[:, sl], P[:])

            fr_store = {}

            def emit_gru_step(step, phase):
                # phase in 0..2*CHUNKS-1: alternating front/back per chunk
                c = phase // 2
                if phase % 2 == 0:
                    fr_store[(step, c)] = emit_gru_front(step, c)
                else:
                    emit_gru_back(step, c, *fr_store.pop((step, c)))

            # ---- main loop, software-pipelined: gates of pair p-1 are
            # interleaved between the xh/S matmul bursts of pair p ----
            for p in range(PAIRS):
                xt = xtp.tile([32, 2, 2, N], fp8)
                (nc.sync if p % 2 == 0 else nc.gpsimd).dma_start(
                    out=xt[:], in_=xT_d[p]
                )
                xh = xhp.tile([128, KT, 128], fp8)

                # 4*CHUNKS gate phases (2 steps x CHUNKS x front/back) spread
                # over the 8 xh groups + S matmul of this pair
                phases = 4 * CHUNKS
                done = 0

                def gates(upto):
                    nonlocal done
                    while done < upto:
                        ph = done
                        step = 2 * (p - 1) + ph // (2 * CHUNKS)
                        emit_gru_step(step, ph % (2 * CHUNKS))
                        done += 1

                for g in range(8):
                    if p >= 1:
                        gates((g + 1) * phases // 8)
                    emit_xh_groups(xt, xh, [g])
                if p >= 1:
                    gates(phases)

                # S-pair matmul: psS[2*64 feat, 512 my-nodes], fp8 DoubleRow
                psS = pssp.tile([128, NS], mybir.dt.float32)
                for k in range(KT // 2):
                    nc.tensor.matmul(
                        psS[:],
                        lhsT=xh[:, 2 * k : 2 * k + 2, :],
                        rhs=adjT_sb[:, 2 * k : 2 * k + 2, :],
                        start=(k == 0), stop=(k == KT // 2 - 1),
                        perf_mode=DR,
                    )
                # stage S for this pair's two steps: [feat, step, node] bf16
                S_sb = ssbp.tile([F_OUT, 2, NS], bf16)
                nc.scalar.copy(S_sb[:, 0, :], psS[0:64, :])
                nc.vector.tensor_copy(S_sb[:, 1, :], psS[64:128, :])
                S_tiles[0] = S_sb
                S_tiles[1] = S_sb

            # drain: gates for the last pair
            for ph in range(4 * CHUNKS):
                step = 2 * (PAIRS - 1) + ph // (2 * CHUNKS)
                emit_gru_step(step, ph % (2 * CHUNKS))

            # gpsimd DMA casts bf16 -> f32 on the way out
            nc.gpsimd.dma_start(out=out_d[:], in_=H[0][:])

    nc.compile()
    return nc


def _prep_inputs(x, adj, fc_w, Wz, Wr, Wh, Lz, Lr, Lh, bz, br, bh):
    bf16 = ml_dtypes.bfloat16
    fp8 = ml_dtypes.float8_e4m3
    f32 = np.float32

    # x [T, N, F] -> [PAIRS, f%32, f//32, step, N] fp8 (DoubleRow K packing)
    xT = np.ascontiguousarray(
        x.reshape(PAIRS, 2, N, 2, 32).transpose(0, 4, 3, 1, 2)
    ).astype(fp8)
    fcw = np.ascontiguousarray(
        (fc_w * FCW_SCALE).reshape(2, 32, H1).transpose(1, 0, 2)
    ).astype(fp8)

    inv = 1.0 / (ADJ_SCALE * FCW_SCALE)

    def fold(W, L):
        return (W.astype(np.float64) @ L[:F_OUT].astype(np.float64)) * inv

    mz, mr, mh = fold(Wz, Lz), fold(Wr, Lr), fold(Wh, Lh)
    # column order [r | -z]: sigmoid rows 0-63 give r, rows 64-127 give
    # z' = 1-z directly (negated z weights + bias)
    mzr = np.concatenate([mr, -mz], axis=1).astype(bf16)        # [64, 128]
    lzr = np.concatenate(
        [Lr[F_OUT:].astype(np.float64), -Lz[F_OUT:].astype(np.float64)], axis=1
    ).astype(bf16)                                              # [64, 128]
    bzr = np.concatenate([br, -bz]).reshape(128, 1).astype(f32)

    shared = {
        "xT": xT, "fcw": fcw,
        "mzr": mzr, "lzr": lzr,
        "mh": mh.astype(bf16), "lh": Lh[F_OUT:].astype(bf16),
        "bzr": bzr, "bh": bh.reshape(F_OUT, 1).astype(f32),
    }
    in_maps = []
    for c in range(NCORES):
        m = dict(shared)
        at = adj[c * NS : (c + 1) * NS, :].T * ADJ_SCALE  # [N, NS]
        m["adjT"] = np.ascontiguousarray(
            at.reshape(KT, 128, NS).transpose(1, 0, 2)
        ).astype(fp8)
        in_maps.append(m)
    return in_maps


def kernel(x, adj, fc_w, fc_b, Wz, Wr, Wh, Lz, Lr, Lh, bz, br, bh):
    x = np.asarray(x, np.float32)
    adj = np.asarray(adj, np.float32)
    args = [np.asarray(a, np.float32) for a in (fc_w, Wz, Wr, Wh, Lz, Lr, Lh, bz, br, bh)]
    fc_b = np.asarray(fc_b, np.float32)
    if np.any(fc_b != 0.0):
        # fc_b can't fold into the per-partition activation bias (it varies
        # along the free dim); the reference always passes zeros. Pure-numpy
        # fallback keeps kernel() correct for arbitrary inputs.
        return _numpy_ref(x, adj, args[0], fc_b, *args[1:])

    from concourse.bass_utils import run_bass_kernel_spmd

    if "nc" not in _cache:
        _cache["nc"] = _build()
    nc = _cache["nc"]

    in_maps = _prep_inputs(x, adj, *args)
    trace = bool(int(os.environ.get("BASS_KERNEL_TRACE", "0")))
    kwargs = {}
    if trace:
        _install_trace_shim()
        tmpdir = os.environ.get("BASS_KERNEL_TRACE_DIR")
        if tmpdir:
            os.makedirs(tmpdir, exist_ok=True)
            kwargs["tmpdir"] = tmpdir
    res = run_bass_kernel_spmd(
        nc, in_maps, core_ids=list(range(NCORES)), trace=trace, **kwargs
    )
    _cache["last_result"] = res

    out = np.empty((1, N, F_OUT), np.float32)
    for c in range(NCORES):
        out[0, c * NS : (c + 1) * NS, :] = res.results[c]["out"].T
    return out


def _install_trace_shim():
    """Register the NTFF profile hook (this image's antenv lacks axon_hooks)
    and stub out the artifact upload so profiling works offline."""
    import types

    try:
        from antenv import axon_hooks  # noqa: F401
        return
    except ImportError:
        pass
    sys.path.insert(0, "/root/.axon_site")
    from trn_agent_boot.trn_boot import _ntff_profile_via_ctypes

    hook = _ntff_profile_via_ctypes("/opt/axon/libaxon_pjrt.so")
    m = types.ModuleType("antenv.axon_hooks")
    m.get_axon_ntff_profile_hook = lambda: hook
    m.set_axon_ntff_profile_hook = lambda h: None
    sys.modules["antenv.axon_hooks"] = m
    import antenv

    antenv.axon_hooks = m
    from concourse import bass_utils as _bu

    _bu.upload_artifacts = lambda tmpdir: tmpdir


def _numpy_ref(x, adj, fc_w, fc_b, Wz, Wr, Wh, Lz, Lr, Lh, bz, br, bh):
    def sigmoid(v):
        return 1.0 / (1.0 + np.exp(-v))

    xh = np.maximum(x @ fc_w + fc_b, 0.0)
    h = np.zeros((N, F_OUT), np.float32)
    for t in range(T):
        s = adj @ xh[t]
        az, ar, ah = s @ Wz, s @ Wr, s @ Wh
        z = sigmoid(np.concatenate([az, h], -1) @ Lz + bz)
        r = sigmoid(np.concatenate([ar, h], -1) @ Lr + br)
        ht = np.tanh(np.concatenate([ah, h * r], -1) @ Lh + bh)
        h = z * h + (1.0 - z) * ht
    return h[None].astype(np.float32)


# revision 14
# speedup vs baseline: 1.3105x; 1.3105x over previous
"""TGCN (dense-graph GRU) Trainium2 kernel, 8-core SPMD, no collectives.

Math (per reference):
  xh_t = relu(x_t @ fc_w + fc_b)                    [N, H]
  S_t  = adj @ xh_t                                 (assoc: adj@(xh@W) = (adj@xh)@W)
  z_t  = sigmoid(S_t @ Mz + h @ Lz_bot + bz)        Mz = Wz @ Lz_top (host-folded)
  r_t  = sigmoid(S_t @ Mr + h @ Lr_bot + br)
  ht_t = tanh   (S_t @ Mh + (h*r) @ Lh_bot + bh)
  h    = z*h + (1-z)*ht = h + (1-z)*(ht - h)

Sharding: row-partition adj across 8 cores (512 nodes each). The GRU cell is
row-local, so each core runs the whole time loop on its shard independently.
x is replicated (each core redundantly computes xh for all nodes).

Measured HW facts this version is built around (from perfetto traces):
  - PE streams ONE 128-deep column per cycle at 2.4GHz when kept busy;
    fp8 DoubleRow streams its 2 k-slices SERIALLY (2N cols per instr), so
    fp8 gives no matmul-throughput win -- only halved instruction count.
    The S matmul is therefore a hard ~6.8us/pair floor.
  - fp8-DR as the xh matmul is LDWEIGHTS-bound (127ns/instr vs bf16's
    29-47ns): xh stays bf16 with the baseline tiny-N shape.
  - Gate matmuls must be ONE K=128 matmul per gate (concat tiles), not
    PSUM-accumulated K=64 pairs (those double PE gate time).

v3 vs the 362us baseline:
  - S matmul in fp8e4 + DoubleRow: 16 instrs/pair instead of 32 (saves the
    per-instruction overhead, ~20-35us). adj scaled by N and fc_w by 16 to
    dodge fp8e4 subnormals; scales folded out of the gate weights.
  - ONE fused sigmoid per step ([128,512]: rows 0-63 z' = 1-z via negated
    z-weights, rows 64-127 r) instead of separate R/Z ACT ops.
  - Combine h_new = h + z'*(ht-h): 4 DVE ops/step (rh, D, P, add) vs 5.
    TensorTensor ops need both INPUTS on the same start partition but the
    OUTPUT is free -- D bridges 64->0, P bridges 0->64.
  - Output leaves via a gpsimd casting DMA (no Hout staging copy).

Dtypes: xh matmul bf16 -> relu casts to fp8 for the S matmul; gates bf16;
h state bf16; PSUM f32.
"""

import os
import sys

sys.path.insert(0, "/opt/trn_rl_repo")

import numpy as np
import ml_dtypes

T, N, F_IN, H1, F_OUT = 48, 4096, 64, 64, 64
NCORES = 8
NS = N // NCORES          # nodes per core = 512
PAIRS = T // 2            # 24
KT = N // 128             # 32 k-tiles for the adj matmul (fp8 DR: 16 instrs)
ADJ_SCALE = float(N)      # adj entries ~1/N are subnormal in fp8e4
FCW_SCALE = 16.0          # fc_w entries ~0.05 land near fp8e4 subnormals

_cache = {}


def _build():
    import concourse.bass as bass
    import concourse.mybir as mybir
    import concourse.tile as tile
    from concourse import bacc

    f32 = mybir.dt.float32
    bf16 = mybir.dt.bfloat16
    fp8 = mybir.dt.float8e4
    AF = mybir.ActivationFunctionType
    DR = mybir.MatmulPerfMode.DoubleRow
    ALU = mybir.AluOpType

    nc = bacc.Bacc(
        "TRN2",
        target_bir_lowering=False,
        debug=False,
        enable_asserts=False,
        num_devices=NCORES,
    )

    # DRAM parameters (per-core shapes)
    adjT_d = nc.dram_tensor("adjT", [128, KT, NS], fp8, kind="ExternalInput").ap()
    xT_d = nc.dram_tensor("xT", [PAIRS, F_IN, 2, N], bf16, kind="ExternalInput").ap()
    fcw_d = nc.dram_tensor("fcw", [F_IN, H1], bf16, kind="ExternalInput").ap()
    wzr_d = nc.dram_tensor("wzr", [128, 128], bf16, kind="ExternalInput").ap()
    wh_d = nc.dram_tensor("wh", [128, F_OUT], bf16, kind="ExternalInput").ap()
    bzr_d = nc.dram_tensor("bzr", [128, 1], f32, kind="ExternalInput").ap()
    bh_d = nc.dram_tensor("bh", [F_OUT, 1], f32, kind="ExternalInput").ap()
    out_d = nc.dram_tensor("out", [F_OUT, NS], f32, kind="ExternalOutput").ap()

    with tile.TileContext(nc) as tc:
        with (
            tc.tile_pool(name="const", bufs=1) as constp,
            tc.tile_pool(name="state", bufs=1) as statep,
            tc.tile_pool(name="xt", bufs=2) as xtp,
            tc.tile_pool(name="xh", bufs=2) as xhp,
            tc.tile_pool(name="gw", bufs=3) as gwp,
            tc.tile_pool(name="psx", bufs=3, space="PSUM") as psxp,
            tc.tile_pool(name="pss", bufs=2, space="PSUM") as pssp,
            tc.tile_pool(name="pszr", bufs=2, space="PSUM") as pszrp,
            tc.tile_pool(name="psh", bufs=1, space="PSUM") as pshp,
        ):
            # ---- constants ----
            # fcw first so pair 0 can start immediately; adjT host-pre-tiled
            fcw_sb = constp.tile([F_IN, H1], bf16)
            nc.sync.dma_start(out=fcw_sb[:], in_=fcw_d[:])
            adjT_sb = constp.tile([128, KT, NS], fp8)
            for q, eng in enumerate((nc.sync, nc.gpsimd, nc.gpsimd, nc.sync)):
                eng.dma_start(
                    out=adjT_sb[:, q * 8 : (q + 1) * 8, :],
                    in_=adjT_d[:, q * 8 : (q + 1) * 8, :],
                )
            wzr_sb = constp.tile([128, 128], bf16)
            wh_sb = constp.tile([128, F_OUT], bf16)
            bzr_sb = constp.tile([128, 1], f32)
            bh_sb = constp.tile([F_OUT, 1], f32)
            for dst, src in (
                (wzr_sb, wzr_d), (wh_sb, wh_d), (bzr_sb, bzr_d), (bh_sb, bh_d),
            ):
                nc.gpsimd.dma_start(out=dst[:], in_=src[:])

            # ---- state ----
            # Concat rhs tiles for the K=128 gate matmuls: rows 0-63 carry
            # S_t.T (refreshed per pair), rows 64-127 the recurrent state:
            # h.T in CzS_*, (h*r).T in ChS_*. h rotates over 4 buffers so
            # per-pair S refreshes never serialize with the chain.
            CzS = []
            ChS = []
            for i in range(4):
                czsi = statep.tile([128, NS], bf16, tag=f"CzS{i}", name=f"CzS{i}")
                chsi = statep.tile([128, NS], bf16, tag=f"ChS{i}", name=f"ChS{i}")
                CzS.append(czsi)
                ChS.append(chsi)
            nc.vector.memset(CzS[0][:], 0.0)

            def emit_xh_groups(xt, xh, groups):
                # xh-pair matmuls: out[128 nodes, 64] = xT_slice.T @ fcw,
                # bf16 (tiny-N matmuls run at ~30-47ns; fp8-DR would be
                # LDWEIGHTS-bound). relu casts f32 PSUM -> fp8 SBUF.
                for g in groups:
                    ps = psxp.tile([128, 512], mybir.dt.float32)
                    for j in range(4):
                        k = 4 * g + j
                        for s in (0, 1):
                            nc.tensor.matmul(
                                ps[:, j * 128 + s * 64 : j * 128 + (s + 1) * 64],
                                lhsT=xt[:, s, k * 128 : (k + 1) * 128],
                                rhs=fcw_sb[:],
                                start=True, stop=True,
                            )
                    dst = xh[:, 4 * g : 4 * (g + 1), :].rearrange("p a b -> p (a b)")
                    if g % 2 == 0:
                        nc.scalar.activation(dst, ps[:], AF.Relu)
                    else:
                        nc.vector.tensor_scalar(dst, ps[:], 0.0, None, ALU.max)

            def emit_gru_front(step):
                # ONE fused sigmoid: rows 0-63 z' = 1-z (negated z weights),
                # rows 64-127 r. Then rh = r * h into the ChS bottom.
                cur = CzS[step % 4]
                ch = ChS[step % 4]
                ps_zr = pszrp.tile([128, NS], mybir.dt.float32, tag="ps_zr")
                nc.tensor.matmul(ps_zr[:], lhsT=wzr_sb[:], rhs=cur[:],
                                 start=True, stop=True)
                ZR = gwp.tile([128, NS], bf16, tag="ZR")
                nc.scalar.activation(ZR[:], ps_zr[:], AF.Sigmoid, bias=bzr_sb[:])
                nc.vector.tensor_mul(ch[64:128, :], ZR[64:128, :], cur[64:128, :])
                return ZR

            def emit_gru_back(step, ZR):
                # ht = tanh(wh.T @ [S; rh]); h_new = h + z'*(ht - h).
                # TensorTensor inputs must share a start partition; outputs
                # are free: D bridges 64->0, P bridges 0->64.
                cur = CzS[step % 4]
                ch = ChS[step % 4]
                nxt = CzS[(step + 1) % 4]
                ps_h = pshp.tile([F_OUT, NS], mybir.dt.float32)
                nc.tensor.matmul(ps_h[:], lhsT=wh_sb[:], rhs=ch[:],
                                 start=True, stop=True)
                HT = gwp.tile([128, NS], bf16, tag="HT")
                nc.scalar.activation(HT[64:128, :], ps_h[:], AF.Tanh,
                                     bias=bh_sb[:])
                D = gwp.tile([128, NS], bf16, tag="D")
                nc.vector.tensor_tensor(D[0:64, :], HT[64:128, :],
                                        cur[64:128, :], ALU.subtract)
                P = gwp.tile([128, NS], bf16, tag="P")
                nc.vector.tensor_mul(P[64:128, :], ZR[0:64, :], D[0:64, :])
                nc.vector.tensor_add(nxt[64:128, :], cur[64:128, :],
                                     P[64:128, :])

            # ---- main loop, software-pipelined: gates of pair p-1 are
            # emitted between the xh/S matmul bursts of pair p so the
            # sequential GRU chain hides under parallel PE work. ----
            for p in range(PAIRS):
                xt = xtp.tile([F_IN, 2, N], bf16)
                (nc.sync if p % 2 == 0 else nc.gpsimd).dma_start(
                    out=xt[:], in_=xT_d[p]
                )
                xh = xhp.tile([128, KT, 128], fp8)

                if p >= 1:
                    zr0 = emit_gru_front(2 * p - 2)
                emit_xh_groups(xt, xh, range(0, 3))
                if p >= 1:
                    emit_gru_back(2 * p - 2, zr0)
                emit_xh_groups(xt, xh, range(3, 6))
                if p >= 1:
                    zr1 = emit_gru_front(2 * p - 1)
                emit_xh_groups(xt, xh, range(6, 8))
                if p >= 1:
                    emit_gru_back(2 * p - 1, zr1)

                # S-pair matmul: psS[2*64 feat, 512 my-nodes], fp8 DoubleRow
                # over adjacent k-tile pairs (16 instrs, 1024 cols each).
                psS = pssp.tile([128, NS], mybir.dt.float32)
                for k in range(KT // 2):
                    nc.tensor.matmul(
                        psS[:],
                        lhsT=xh[:, 2 * k : 2 * k + 2, :],
                        rhs=adjT_sb[:, 2 * k : 2 * k + 2, :],
                        start=(k == 0), stop=(k == KT // 2 - 1),
                        perf_mode=DR,
                    )

                # refresh concat tops for this pair's two steps; the 4-way
                # rotation keeps these copies off the sequential gate chain
                s0, s1 = (2 * p) % 4, (2 * p + 1) % 4
                nc.scalar.copy(CzS[s0][0:64, :], psS[0:64, :])
                nc.vector.tensor_copy(ChS[s0][0:64, :], psS[0:64, :])
                nc.scalar.copy(CzS[s1][0:64, :], psS[64:128, :])
                nc.vector.tensor_copy(ChS[s1][0:64, :], psS[64:128, :])

            # drain: gates for the last pair
            zr = emit_gru_front(2 * PAIRS - 2)
            emit_gru_back(2 * PAIRS - 2, zr)
            zr = emit_gru_front(2 * PAIRS - 1)
            emit_gru_back(2 * PAIRS - 1, zr)

            # gpsimd DMA casts bf16 -> f32 on the way out
            nc.gpsimd.dma_start(out=out_d[:],
                                in_=CzS[(2 * PAIRS) % 4][64:128, :])

    nc.compile()
    return nc


def _prep_inputs(x, adj, fc_w, Wz, Wr, Wh, Lz, Lr, Lh, bz, br, bh):
    bf16 = ml_dtypes.bfloat16
    fp8 = ml_dtypes.float8_e4m3
    f32 = np.float32

    # x [T, N, F] -> [PAIRS, F, step, N] (features on partitions), bf16
    xT = np.ascontiguousarray(
        x.reshape(PAIRS, 2, N, F_IN).transpose(0, 3, 1, 2)
    ).astype(bf16)
    fcw = (fc_w * FCW_SCALE).astype(bf16)

    inv = 1.0 / (ADJ_SCALE * FCW_SCALE)

    def fold(W, L):
        return (W.astype(np.float64) @ L[:F_OUT].astype(np.float64)) * inv

    mz, mr, mh = fold(Wz, Lz), fold(Wr, Lr), fold(Wh, Lh)
    # stacked [K=128] weights: rows 0-63 hit S_t, rows 64-127 hit h / (h*r).
    # Column order [-z | r]: the fused sigmoid yields z' = 1-z in rows 0-63
    # and r in rows 64-127.
    wzr = np.concatenate(
        [
            np.concatenate([-mz, mr], axis=1),
            np.concatenate([-Lz[F_OUT:].astype(np.float64),
                            Lr[F_OUT:].astype(np.float64)], axis=1),
        ],
        axis=0,
    ).astype(bf16)                                              # [128, 128]
    wh = np.concatenate([mh, Lh[F_OUT:].astype(np.float64)], axis=0).astype(
        bf16
    )                                                           # [128, 64]
    bzr = np.concatenate([-bz, br]).reshape(128, 1).astype(f32)

    shared = {
        "xT": xT, "fcw": fcw, "wzr": wzr, "wh": wh,
        "bzr": bzr, "bh": bh.reshape(F_OUT, 1).astype(f32),
    }
    in_maps = []
    for c in range(NCORES):
        m = dict(shared)
        at = adj[c * NS : (c + 1) * NS, :].T * ADJ_SCALE  # [N, NS]
        m["adjT"] = np.ascontiguousarray(
            at.reshape(KT, 128, NS).transpose(1, 0, 2)
        ).astype(fp8)
        in_maps.append(m)
    return in_maps


def kernel(x, adj, fc_w, fc_b, Wz, Wr, Wh, Lz, Lr, Lh, bz, br, bh):
    x = np.asarray(x, np.float32)
    adj = np.asarray(adj, np.float32)
    args = [np.asarray(a, np.float32) for a in (fc_w, Wz, Wr, Wh, Lz, Lr, Lh, bz, br, bh)]
    fc_b = np.asarray(fc_b, np.float32)
    if np.any(fc_b != 0.0):
        # fc_b can't fold into the per-partition activation bias (it varies
        # along the free dim); the reference always passes zeros. Pure-numpy
        # fallback keeps kernel() correct for arbitrary inputs.
        return _numpy_ref(x, adj, args[0], fc_b, *args[1:])

    from concourse.bass_utils import run_bass_kernel_spmd

    if "nc" not in _cache:
        _cache["nc"] = _build()
    nc = _cache["nc"]

    in_maps = _prep_inputs(x, adj, *args)
    trace = bool(int(os.environ.get("BASS_KERNEL_TRACE", "0")))
    kwargs = {}
    if trace:
        _install_trace_shim()
        tmpdir = os.environ.get("BASS_KERNEL_TRACE_DIR")
        if tmpdir:
            os.makedirs(tmpdir, exist_ok=True)
            kwargs["tmpdir"] = tmpdir
    res = run_bass_kernel_spmd(
        nc, in_maps, core_ids=list(range(NCORES)), trace=trace, **kwargs
    )
    _cache["last_result"] = res

    out = np.empty((1, N, F_OUT), np.float32)
    for c in range(NCORES):
        out[0, c * NS : (c + 1) * NS, :] = res.results[c]["out"].T
    return out


def _install_trace_shim():
    """Register the NTFF profile hook (this image's antenv lacks axon_hooks)
    and stub out the artifact upload so profiling works offline."""
    import types

    try:
        from antenv import axon_hooks  # noqa: F401
        return
    except ImportError:
        pass
    sys.path.insert(0, "/root/.axon_site")
    from trn_agent_boot.trn_boot import _ntff_profile_via_ctypes

    hook = _ntff_profile_via_ctypes("/opt/axon/libaxon_pjrt.so")
    m = types.ModuleType("antenv.axon_hooks")
    m.get_axon_ntff_profile_hook = lambda: hook
    m.set_axon_ntff_profile_hook = lambda h: None
    sys.modules["antenv.axon_hooks"] = m
    import antenv

    antenv.axon_hooks = m
    from concourse import bass_utils as _bu

    _bu.upload_artifacts = lambda tmpdir: tmpdir


def _numpy_ref(x, adj, fc_w, fc_b, Wz, Wr, Wh, Lz, Lr, Lh, bz, br, bh):
    def sigmoid(v):
        return 1.0 / (1.0 + np.exp(-v))

    xh = np.maximum(x @ fc_w + fc_b, 0.0)
    h = np.zeros((N, F_OUT), np.float32)
    for t in range(T):
        s = adj @ xh[t]
        az, ar, ah = s @ Wz, s @ Wr, s @ Wh
        z = sigmoid(np.concatenate([az, h], -1) @ Lz + bz)
        r = sigmoid(np.concatenate([ar, h], -1) @ Lr + br)
        ht = np.tanh(np.concatenate([ah, h * r], -1) @ Lh + bh)
        h = z * h + (1.0 - z) * ht
    return h[None].astype(np.float32)
